# revision 13
# baseline (speedup 1.0000x reference)
"""Trainium2 Bass kernel for nn_FFTAppearanceEncoder.

Data-parallel over the 32 frames: 4 frames per NeuronCore x 8 cores.
Per frame on device:
  - pixel stream: conv1 as a single K=54 tap-packed matmul (27 pre-shifted
    input copies, both streams share the matmul), conv2..conv5 as
    PSUM-accumulated tap matmuls (dy-pairs packed into K=128 via a
    one-row-shifted partition copy), eval-BN folded into weights, bias+ReLU
    fused into the ACT eviction, 2x2 maxpool fused into the PSUM eviction
    on DVE.
  - FFT stream: fft2(ortho)+fftshift as DFT-matrix matmuls (shift and norm
    folded into host-built cos/sin matrices) with PE transposes between the
    row and column passes; log1p(|.|) on ACT (batched so the sqrt/ln table
    sets each load once).
  - ROI pooling + scene mean as matmuls against host-built {0,1} box masks,
    exact mask/area scaling applied as a per-partition fp32 multiply.
  - fused head matmul + LayerNorm + ReLU on device.
All activations/weights bf16, accumulation fp32 (PSUM), head in fp32.
"""

import os
import sys

import numpy as np

if "/opt/trn_rl_repo" not in sys.path and os.path.isdir("/opt/trn_rl_repo"):
    sys.path.insert(0, "/opt/trn_rl_repo")

from ml_dtypes import bfloat16

import concourse.bass as bass
import concourse.tile as tile
from concourse import mybir
from concourse.bass_utils import run_bass_kernel_spmd
from concourse.vector_clock import ScopedClock

# ---------------------------------------------------------------- constants
BN_EPS = 1e-5
LN_EPS = 1e-5
B, T, M = 4, 8, 8
N_FRAMES = B * T
N_CORES = 8
FPC = N_FRAMES // N_CORES  # frames per core
H = W = 128
HP1 = 130  # padded 128x128 plane row stride
SZ1 = HP1 * HP1  # 16900
HP2 = 66  # padded 64x64
SZ2 = HP2 * HP2  # 4356
HP3 = 34  # padded 32x32
SZ3 = HP3 * HP3  # 1156
BF = mybir.dt.bfloat16
F32 = mybir.dt.float32
AF = mybir.ActivationFunctionType
MAX = mybir.AluOpType.max
MULT = mybir.AluOpType.mult
ADD = mybir.AluOpType.add
SUB = mybir.AluOpType.subtract

# ------------------------------------------------------- tile drain patch
# walrus in this toolchain rejects >1 sync-wait on the TileContext final
# drain; split the accumulated waits across several drain instructions.


def _patched_drain_and_barrier(self, tick_clock, wait_clock):
    nc = self.nc
    drain_inst = nc.sync.drain()
    wait_clock.add_sem_waits(
        drain_inst.ins, ScopedClock({None: tick_clock.global_clock})
    )
    si = drain_inst.ins.sync_info
    waits = list(si.on_wait)
    if len(waits) > 1:
        drain_inst.ins.sync_info = mybir.SyncInfo(
            on_wait=waits[:1], on_update=list(si.on_update)
        )
        for i in range(1, len(waits)):
            d2 = nc.sync.drain()
            d2.ins.sync_info = mybir.SyncInfo(on_wait=[waits[i]], on_update=[])
    nc.all_engine_barrier()
    assert self.sems is not None
    popped = nc._tile_sem_poison_stack.pop()
    assert popped is self._sem_poison
    nc.clear_and_free_semaphores(list(self.sems.allocated().values()))
    nc.all_engine_barrier()


tile.TileContext._drain_and_barrier = _patched_drain_and_barrier


# ---------------------------------------------------------------- host prep
def _fold_bn(p):
    g = np.asarray(p["g"], np.float32)
    v = np.asarray(p["v"], np.float32)
    b = np.asarray(p["b"], np.float32)
    m = np.asarray(p["m"], np.float32)
    beta = np.asarray(p["beta"], np.float32)
    s = g / np.sqrt(v + BN_EPS)
    wf = np.asarray(p["w"], np.float32) * s[:, None, None, None]
    t = (b - m) * s + beta
    return wf.astype(np.float32), t.astype(np.float32)


def _dft_mats():
    k = np.arange(128)
    ksh = (k + 64) % 128
    ang_row = 2.0 * np.pi * np.outer(ksh, k) / 128.0
    cr = (np.cos(ang_row) / 128.0).astype(np.float32)
    ci = (-np.sin(ang_row) / 128.0).astype(np.float32)
    ang_col = 2.0 * np.pi * np.outer(k, ksh) / 128.0  # [w, l_shifted]
    gc = np.cos(ang_col).astype(np.float32)
    gs = np.sin(ang_col).astype(np.float32)
    return cr.T.copy(), ci.T.copy(), gc, gs, (-gs).copy()


def _roi_masks(boxes_n, dmask):
    """boxes_n [M,4] fp32, dmask [M] -> K01 [M,1024] {0,1}, scale [M]"""
    Hf = Wf = 32
    cx = (boxes_n[:, 0] * Wf).astype(np.float32)
    cy = (boxes_n[:, 1] * Hf).astype(np.float32)
    bw = np.maximum(boxes_n[:, 2] * Wf, np.float32(2.0)).astype(np.float32)
    bh = np.maximum(boxes_n[:, 3] * Hf, np.float32(2.0)).astype(np.float32)
    x1 = np.clip((cx - bw / 2).astype(np.int32), 0, Wf - 1)
    y1 = np.clip((cy - bh / 2).astype(np.int32), 0, Hf - 1)
    x2 = np.maximum(np.clip((cx + bw / 2).astype(np.int32), 1, Wf), x1 + 1)
    y2 = np.maximum(np.clip((cy + bh / 2).astype(np.int32), 1, Hf), y1 + 1)
    rows = np.arange(Hf)[None, :]
    cols = np.arange(Wf)[None, :]
    rm = ((rows >= y1[:, None]) & (rows < y2[:, None])).astype(np.float32)
    cm = ((cols >= x1[:, None]) & (cols < x2[:, None])).astype(np.float32)
    K01 = (rm[:, :, None] * cm[:, None, :]).reshape(M, -1)
    area = rm.sum(-1) * cm.sum(-1)
    scale = (dmask >= 0.5).astype(np.float32) / area
    return K01, scale.astype(np.float32)


def _wt(w):  # [cout, cin] -> [cin, cout]
    return np.ascontiguousarray(w.T)


def _pack6(wf, inverted=False):
    # [128, 6, cout]: slots 0-2: (dy0 on A-half rows, dy1 on B-half rows)
    # per dx; slots 3-5: dy2 on B-half rows, zeros on A-half.
    cout = wf.shape[0]
    out = np.zeros((128, 6, cout), np.float32)
    a, b = slice(0, 64), slice(64, 128)
    if inverted:
        a, b = b, a
    for dx in range(3):
        out[a, dx, :] = _wt(wf[:, :, 0, dx])
        out[b, dx, :] = _wt(wf[:, :, 1, dx])
        out[b, 3 + dx, :] = _wt(wf[:, :, 2, dx])
    return out


def _pack9(wf):
    cout, cin = wf.shape[0], wf.shape[1]
    out = np.zeros((128, 9, cout), np.float32)
    for dy in range(3):
        for dx in range(3):
            out[:cin, 3 * dy + dx, :] = _wt(wf[:, :, dy, dx])
    return out


def _pack_weights(params):
    """All constant (per-core-identical) device inputs derived from params."""
    w1p, t1p = _fold_bn(params["pix"][0])
    w2p, t2p = _fold_bn(params["pix"][1])
    w3p, t3p = _fold_bn(params["pix"][2])
    w4p, t4p = _fold_bn(params["pix"][3])
    w5p, t5p = _fold_bn(params["pix"][4])
    w1f, t1f = _fold_bn(params["fft"][0])
    w2f, t2f = _fold_bn(params["fft"][1])
    w3f, t3f = _fold_bn(params["fft"][2])

    # conv1 (both streams): lhsT [54, 128]; partition c*9+3dy+dx
    wc1 = np.zeros((54, 128), np.float32)
    for c in range(3):
        for dy in range(3):
            for dx in range(3):
                p = c * 9 + 3 * dy + dx
                wc1[p, 0:64] = w1p[:, c, dy, dx]
                wc1[27 + p, 64:128] = w1f[:, c, dy, dx]

    biases = np.zeros((128, 9), np.float32)
    biases[0:64, 0] = t1p
    biases[64:128, 0] = t1f
    biases[0:64, 1] = t2p
    biases[0:128, 2] = t3p
    biases[0:128, 3] = t4p
    biases[0:128, 4] = t5p[0:128]
    biases[0:128, 5] = t5p[128:256]
    biases[0:128, 6] = t2f
    biases[0:128, 7] = t3f[0:128]
    biases[0:128, 8] = t3f[128:256]

    crT, ciT, gc, gs, gsn = _dft_mats()
    dft = np.stack([crT, ciT, gc, gs, gsn], axis=1)  # [128, 5, 128]
    ident = np.eye(128, dtype=np.float32)

    fuse_w = np.asarray(params["fuse_w"], np.float32)  # [256, 768]
    fwT = np.ascontiguousarray(fuse_w.T).reshape(6, 128, 256).transpose(1, 0, 2)
    fwT = np.ascontiguousarray(fwT)  # [128, 6, 256]
    fb_row = np.asarray(params["fuse_b"], np.float32).reshape(1, 256)
    ones1 = np.ones((1, 32), np.float32)
    lng = np.broadcast_to(np.asarray(params["ln_g"], np.float32), (32, 256)).copy()
    lnb = np.broadcast_to(np.asarray(params["ln_b"], np.float32), (32, 256)).copy()
    eps = np.full((32, 1), LN_EPS, np.float32)

    def bf(a):
        return np.ascontiguousarray(a).astype(bfloat16)

    return {
        "wc1": bf(wc1),
        "wc2": bf(_pack6(w2p)),
        "wc3": bf(_pack6(w3p)),
        "wc4": bf(_pack9(w4p)),
        "wc5": bf(_pack9(w5p)),
        "wg1": bf(_pack6(w2f, inverted=True)),
        "wg2": bf(_pack9(w3f)),
        "biases": biases,
        "dft": bf(dft),
        "ident": bf(ident),
        "fwT": bf(fwT),
        "fb_row": bf(fb_row),
        "ones1": bf(ones1),
        "lng": lng,
        "lnb": lnb,
        "eps": eps,
    }


def _prep_frames(frames, boxes_n, dmask):
    """Per-frame device inputs. frames [N,3,128,128] fp32."""
    N = frames.shape[0]
    xp = np.pad(frames, ((0, 0), (0, 0), (1, 1), (1, 1)))  # [N,3,130,130]
    xpad = xp.reshape(N, 3, SZ1)
    x27 = np.zeros((N, 27, SZ1), bfloat16)
    for c in range(3):
        for dy in range(3):
            for dx in range(3):
                p = c * 9 + 3 * dy + dx
                off = dy * HP1 + dx
                x27[:, p, : SZ1 - off] = xpad[:, c, off:].astype(bfloat16)
    x_hw = (
        np.ascontiguousarray(frames.transpose(0, 2, 1, 3))
        .reshape(N, 128, 384)
        .astype(bfloat16)
    )
    k01 = np.zeros((N, 1024, 16), bfloat16)
    scales = np.zeros((N, 24), np.float32)
    for n in range(N):
        K01, sc = _roi_masks(boxes_n[n], dmask[n])
        k01[n, :, 0:8] = K01.T.astype(bfloat16)
        k01[n, :, 8:16] = 1.0
        scales[n, 0:8] = sc
        scales[n, 8:16] = 1.0 / 1024.0
        scales[n, 16:24] = sc
    return x27, x_hw, k01, scales


# ------------------------------------------------------------- bass program
def _plane(t, hp):
    """View a [P, hp*hp] plane tile as [P, hp, hp]."""
    return t[:].rearrange("p (r c) -> p r c", c=hp)


def _build_program():
    nc = bass.Bass()

    def din(name, shape, dt=BF):
        return nc.declare_dram_parameter(name, list(shape), dt, isOutput=False)

    d = {
        "x27": din("x27", (FPC, 27, SZ1)),
        "x_hw": din("x_hw", (FPC, 128, 384)),
        "k01": din("k01", (FPC, 1024, 16)),
        "scales": din("scales", (24, FPC), F32),
        "wc1": din("wc1", (54, 128)),
        "wc2": din("wc2", (128, 6, 64)),
        "wc3": din("wc3", (128, 6, 128)),
        "wc4": din("wc4", (128, 9, 128)),
        "wc5": din("wc5", (128, 9, 256)),
        "wg1": din("wg1", (128, 6, 128)),
        "wg2": din("wg2", (128, 9, 256)),
        "biases": din("biases", (128, 9), F32),
        "dft": din("dft", (128, 5, 128)),
        "ident": din("ident", (128, 128)),
        "fwT": din("fwT", (128, 6, 256)),
        "fb_row": din("fb_row", (1, 256)),
        "ones1": din("ones1", (1, 32)),
        "lng": din("lng", (32, 256), F32),
        "lnb": din("lnb", (32, 256), F32),
        "eps": din("eps", (32, 1), F32),
        "out": nc.declare_dram_parameter("out", [32, 256], F32, isOutput=True),
    }

    with tile.TileContext(nc) as tc:
        _emit(nc, tc, d)
    _split_sync_waits(nc)
    return nc


def _split_sync_waits(nc):
    """walrus here allows only one sync-wait per instruction: hoist extra
    waits onto same-engine no-ops inserted just before the instruction."""
    cnt = 0
    for fn in nc.m.functions:
        for bb in fn.blocks:
            il = bb.instructions
            i = 0
            while i < len(il):
                inst = il[i]
                si = inst.sync_info
                if si is None:
                    i += 1
                    continue
                waits = list(si.on_wait)
                if len(waits) > 1:
                    inst.sync_info = mybir.SyncInfo(
                        on_wait=waits[-1:], on_update=list(si.on_update)
                    )
                    for w in waits[:-1]:
                        nop = mybir.InstNoOp(name=f"waitnop-{cnt}")
                        cnt += 1
                        nop.engine = inst.engine
                        nop.sync_info = mybir.SyncInfo(on_wait=[w], on_update=[])
                        il.insert(i, nop)
                        i += 1
                i += 1


def _emit(nc, tc, d):
    import contextlib

    ctx = contextlib.ExitStack()
    with ctx:
        sg = ctx.enter_context(tc.tile_pool(name="sg", bufs=1))
        tp = ctx.enter_context(tc.tile_pool(name="tp", bufs=3))
        pp = ctx.enter_context(tc.tile_pool(name="pp", bufs=2, space="PSUM"))
        sp = ctx.enter_context(tc.tile_pool(name="sp", bufs=1, space="PSUM"))
        XAX = mybir.AxisListType.X

        def pool_evict(ps, psl, rows, w2, out_ap, bias_ap):
            """2x2 maxpool of psum [*, rows, 2*w2] + bias + relu -> out_ap."""
            pv = ps[:].rearrange("p r (w two) -> p r w two", two=2)
            m1 = tp.tile([128, rows, w2], F32, tag=f"m1_{rows}")
            nc.vector.tensor_reduce(out=m1[psl], in_=pv[psl], axis=XAX, op=MAX)
            m1v = m1[:].rearrange("p (r2 two) w -> p r2 w two", two=2)
            m2 = tp.tile([128, rows // 2, w2], F32, tag=f"m2_{rows}")
            nc.vector.tensor_reduce(out=m2[psl], in_=m1v[psl], axis=XAX, op=MAX)
            nc.scalar.activation(out=out_ap, in_=m2[psl], func=AF.Relu, bias=bias_ap)

        # ---------------- constants into SBUF
        def load(name, shape, dt=BF):
            t = sg.tile(list(shape), dt, tag=name)
            nc.sync.dma_start(out=t[:], in_=d[name][:])
            return t

        wc1 = load("wc1", (54, 128))
        wc2 = load("wc2", (128, 6, 64))
        wc3 = load("wc3", (128, 6, 128))
        wc4 = load("wc4", (128, 9, 128))
        wc5 = load("wc5", (128, 9, 256))
        wg1 = load("wg1", (128, 6, 128))
        wg2 = load("wg2", (128, 9, 256))
        bias = load("biases", (128, 9), F32)
        dft = load("dft", (128, 5, 128))
        ident = load("ident", (128, 128))
        fwT = load("fwT", (128, 6, 256))
        fb = load("fb_row", (1, 256))
        ones1 = load("ones1", (1, 32))
        lng = load("lng", (32, 256), F32)
        lnb = load("lnb", (32, 256), F32)
        eps = load("eps", (32, 1), F32)
        scales = load("scales", (24, FPC), F32)
        xhw_all = sg.tile([128, FPC, 384], BF, tag="xhw")
        nc.sync.dma_start(
            out=xhw_all[:], in_=d["x_hw"][:].rearrange("f h c -> h f c")
        )

        # ---------------- working planes (persistent, zero borders)
        c1in = sg.tile([54, SZ1], BF, tag="c1in")
        l1 = sg.tile([128, SZ1], BF, tag="l1")
        p3in = sg.tile([128, SZ2], BF, tag="p3in")
        p4in = sg.tile([128, SZ2], BF, tag="p4in")
        p5in = sg.tile([128, SZ3], BF, tag="p5in")
        g1in = sg.tile([128, SZ2], BF, tag="g1in")
        g2in = sg.tile([128, SZ3], BF, tag="g2in")
        pixmap = sg.tile([128, 2, 1024], BF, tag="pixmap")
        fftmap = sg.tile([128, 2, 1024], BF, tag="fftmap")
        s_all = sg.tile([128, FPC, 384], F32, tag="s_all")
        mag = sg.tile([128, FPC, 384], F32, tag="mag")
        f_all = sg.tile([128, FPC, 384], BF, tag="f_all")
        catT = sg.tile([128, 6, 32], BF, tag="catT")
        y_sb = sg.tile([32, 256], F32, tag="y_sb")

        for t in (c1in, l1, p3in, p4in, p5in, g1in, g2in):
            nc.vector.memset(t[:], 0.0)

        # ================= FFT front: all frames (batches sqrt/ln sets)
        ar = sg.tile([128, 384], BF, tag="ar")
        ai = sg.tile([128, 384], BF, tag="ai")
        arT = sg.tile([128, 384], BF, tag="arT")
        aiT = sg.tile([128, 384], BF, tag="aiT")
        for f in range(FPC):
            xh = xhw_all[:, f, :]
            ps_ar = sp.tile([128, 384], F32, tag="psA")
            ps_ai = sp.tile([128, 384], F32, tag="psB")
            nc.tensor.matmul(ps_ar[:], dft[:, 0, :], xh, start=True, stop=True)
            nc.tensor.matmul(ps_ai[:], dft[:, 1, :], xh, start=True, stop=True)
            nc.scalar.copy(ar[:], ps_ar[:])
            nc.scalar.copy(ai[:], ps_ai[:])
            for c in range(3):
                cs = slice(128 * c, 128 * (c + 1))
                pt = sp.tile([128, 128], BF, tag="psA")
                nc.tensor.transpose(pt[:], ar[:, cs], ident[:])
                nc.scalar.copy(arT[:, cs], pt[:])
                pt2 = sp.tile([128, 128], BF, tag="psB")
                nc.tensor.transpose(pt2[:], ai[:, cs], ident[:])
                nc.scalar.copy(aiT[:, cs], pt2[:])
            for c in range(3):
                cs = slice(128 * c, 128 * (c + 1))
                ps_br = sp.tile([128, 128], F32, tag="psA")
                ps_bi = sp.tile([128, 128], F32, tag="psB")
                nc.tensor.matmul(ps_br[:], arT[:, cs], dft[:, 2, :], start=True, stop=False)
                nc.tensor.matmul(ps_br[:], aiT[:, cs], dft[:, 3, :], start=False, stop=True)
                nc.tensor.matmul(ps_bi[:], aiT[:, cs], dft[:, 2, :], start=True, stop=False)
                nc.tensor.matmul(ps_bi[:], arT[:, cs], dft[:, 4, :], start=False, stop=True)
                sl = s_all[:, f, cs]
                tq = tp.tile([128, 128], F32, tag="sqt")
                nc.scalar.activation(sl, ps_br[:], AF.Square)
                nc.scalar.activation(tq[:], ps_bi[:], AF.Square)
                nc.vector.tensor_tensor(out=sl, in0=sl, in1=tq[:], op=ADD)
        # mag = sqrt(s); f = ln(1 + mag)  (one table load each)
        nc.scalar.activation(mag[:], s_all[:], AF.Sqrt)
        nc.scalar.activation(f_all[:], mag[:], AF.Ln, bias=1.0, scale=1.0)

        # ================= per-frame conv pipeline
        for f in range(FPC):
            # ---- conv1 inputs: x27 DMA + f27 shifted copies
            nc.sync.dma_start(out=c1in[0:27, :], in_=d["x27"][f])
            fa = f_all[:].rearrange("p f (c w) -> p f c w", c=3)
            c1v = _plane(c1in, HP1)
            for c in range(3):
                for dy in range(3):
                    for dx in range(3):
                        p = 27 + c * 9 + 3 * dy + dx
                        r0, s0 = max(0, dy - 1), max(0, dx - 1)
                        nr, ns = 128 - r0, 128 - s0
                        rr0, ss0 = r0 + 1 - dy, s0 + 1 - dx
                        nc.sync.dma_start(
                            out=c1v[p : p + 1, rr0 : rr0 + nr, ss0 : ss0 + ns],
                            in_=fa[r0 : r0 + nr, f, c, s0 : s0 + ns],
                        )
            # ---- conv1: 16 chunks of 8 rows, K=54 -> [128, 1024]
            l1v = _plane(l1, HP1)
            g1v = _plane(g1in, HP2)
            p3v = _plane(p3in, HP2)
            for ch in range(32):
                h0 = 4 * ch
                ps = pp.tile([128, 4, 128], F32, tag="mm")
                nc.tensor.matmul(
                    ps[:], wc1[:], c1v[:, h0 : h0 + 4, 0:128], start=True, stop=True
                )
                nc.scalar.activation(
                    out=l1v[0:64, h0 + 1 : h0 + 5, 1:129],
                    in_=ps[0:64],
                    func=AF.Relu,
                    bias=bias[0:64, 0:1],
                )
                # fft half: maxpool from psum -> g1in A-half (parts 64-127)
                pool_evict(
                    ps,
                    slice(64, 128),
                    4,
                    64,
                    g1v[64:128, 2 * ch + 1 : 2 * ch + 3, 1:65],
                    bias[64:128, 0:1],
                )
            # L1 B-half: shifted dup (B = A + HP1)
            nc.sync.dma_start(
                out=l1[64:128, 0 : SZ1 - HP1], in_=l1[0:64, HP1:SZ1]
            )
            # g1in B-half (parts 0-63) = A-half (parts 64-127) + HP2
            nc.sync.dma_start(
                out=g1in[0:64, 0 : SZ2 - HP2], in_=g1in[64:128, HP2:SZ2]
            )

            # ---- conv2: 16 chunks x 6 slots, K=128 -> [64, 1024]; pool -> p3in
            for ch in range(32):
                h0 = 4 * ch
                ps = pp.tile([128, 4, 128], F32, tag="mm")
                for s in range(6):
                    dxo = s % 3
                    r_extra = 1 if s >= 3 else 0
                    rhs = l1v[:, h0 + r_extra : h0 + r_extra + 4, dxo : dxo + 128]
                    nc.tensor.matmul(
                        ps[0:64], wc2[:, s, :], rhs, start=(s == 0), stop=(s == 5)
                    )
                pool_evict(
                    ps,
                    slice(0, 64),
                    4,
                    64,
                    p3v[0:64, 2 * ch + 1 : 2 * ch + 3, 1:65],
                    bias[0:64, 1:2],
                )
            # p3in B-half dup
            nc.sync.dma_start(
                out=p3in[64:128, 0 : SZ2 - HP2], in_=p3in[0:64, HP2:SZ2]
            )

            # ---- conv3 + conv4 (+ pool) ; g1 (+ pool)
            p4v = _plane(p4in, HP2)
            p5v = _plane(p5in, HP3)
            g2v = _plane(g2in, HP3)
            for ch in range(8):
                h0 = 8 * ch
                ps = pp.tile([128, 8, 64], F32, tag="mm")
                for s in range(6):
                    dxo = s % 3
                    r_extra = 1 if s >= 3 else 0
                    rhs = p3v[:, h0 + r_extra : h0 + r_extra + 8, dxo : dxo + 64]
                    nc.tensor.matmul(
                        ps[:], wc3[:, s, :], rhs, start=(s == 0), stop=(s == 5)
                    )
                nc.scalar.activation(
                    out=p4v[:, h0 + 1 : h0 + 9, 1:65],
                    in_=ps[:],
                    func=AF.Relu,
                    bias=bias[:, 2:3],
                )
            for ch in range(8):
                h0 = 8 * ch
                ps = pp.tile([128, 8, 64], F32, tag="mm")
                for s in range(9):
                    dy, dx = divmod(s, 3)
                    rhs = p4v[:, h0 + dy : h0 + dy + 8, dx : dx + 64]
                    nc.tensor.matmul(
                        ps[:], wc4[:, s, :], rhs, start=(s == 0), stop=(s == 8)
                    )
                pool_evict(
                    ps,
                    slice(0, 128),
                    8,
                    32,
                    p5v[:, 4 * ch + 1 : 4 * ch + 5, 1:33],
                    bias[:, 3:4],
                )
            for ch in range(8):
                h0 = 8 * ch
                ps = pp.tile([128, 8, 64], F32, tag="mm")
                for s in range(6):
                    dxo = s % 3
                    r_extra = 1 if s >= 3 else 0
                    rhs = g1v[:, h0 + r_extra : h0 + r_extra + 8, dxo : dxo + 64]
                    nc.tensor.matmul(
                        ps[:], wg1[:, s, :], rhs, start=(s == 0), stop=(s == 5)
                    )
                pool_evict(
                    ps,
                    slice(0, 128),
                    8,
                    32,
                    g2v[:, 4 * ch + 1 : 4 * ch + 5, 1:33],
                    bias[:, 6:7],
                )

            # ---- conv5 / g2: N=1024 whole map, 9 taps, 2 cout blocks
            for src, wts, outm, bcol in (
                (p5v, wc5, pixmap, 4),
                (g2v, wg2, fftmap, 7),
            ):
                for blk in range(2):
                    for ch in range(2):
                        h0 = 16 * ch
                        ps = pp.tile([128, 16, 32], F32, tag="mm")
                        for s in range(9):
                            dy, dx = divmod(s, 3)
                            rhs = src[:, h0 + dy : h0 + dy + 16, dx : dx + 32]
                            nc.tensor.matmul(
                                ps[:],
                                wts[:, s, 128 * blk : 128 * (blk + 1)],
                                rhs,
                                start=(s == 0),
                                stop=(s == 8),
                            )
                        nc.scalar.activation(
                            out=outm[:, blk, 512 * ch : 512 * (ch + 1)],
                            in_=ps[:].rearrange("p a b -> p (a b)"),
                            func=AF.Relu,
                            bias=bias[:, bcol + blk : bcol + blk + 1],
                        )

            # ---- ROI: featT transposes + mask matmuls
            k01 = tp.tile([128, 8, 16], BF, tag="k01")
            nc.sync.dma_start(
                out=k01[:], in_=d["k01"][f].rearrange("(q p) n -> p q n", p=128)
            )
            ps_rp = sp.tile([16, 256], F32, tag="roiP")
            ps_rf = sp.tile([8, 256], F32, tag="roiF")
            for q in range(8):
                qs = slice(128 * q, 128 * (q + 1))
                ft_p = tp.tile([128, 2, 128], BF, tag="ftp")
                ft_f = tp.tile([128, 2, 128], BF, tag="ftf")
                for cb in range(2):
                    ptp = sp.tile([128, 128], BF, tag="psA")
                    nc.tensor.transpose(ptp[:], pixmap[:, cb, qs], ident[:])
                    nc.scalar.copy(ft_p[:, cb, :], ptp[:])
                    ptf = sp.tile([128, 128], BF, tag="psB")
                    nc.tensor.transpose(ptf[:], fftmap[:, cb, qs], ident[:])
                    nc.scalar.copy(ft_f[:, cb, :], ptf[:])
                nc.tensor.matmul(
                    ps_rp[:],
                    k01[:, q, 0:16],
                    ft_p[:].rearrange("p a b -> p (a b)"),
                    start=(q == 0),
                    stop=(q == 7),
                )
                nc.tensor.matmul(
                    ps_rf[:],
                    k01[:, q, 0:8],
                    ft_f[:].rearrange("p a b -> p (a b)"),
                    start=(q == 0),
                    stop=(q == 7),
                )
            cat_p = tp.tile([16, 256], BF, tag="catp")
            cat_f = tp.tile([8, 256], BF, tag="catf")
            nc.vector.tensor_scalar(
                out=cat_p[:],
                in0=ps_rp[:],
                scalar1=scales[0:16, f : f + 1],
                scalar2=None,
                op0=MULT,
            )
            nc.vector.tensor_scalar(
                out=cat_f[:],
                in0=ps_rf[:],
                scalar1=scales[0:8, f : f + 1],
                scalar2=None,
                op0=MULT,
            )
            # cat transposes into catT [128, 6, 32]
            for cb in range(2):
                cbs = slice(128 * cb, 128 * (cb + 1))
                pt = sp.tile([128, 16], BF, tag="psA")
                nc.tensor.transpose(pt[:], cat_p[:, cbs], ident[0:16, 0:16])
                nc.scalar.copy(catT[:, cb, 8 * f : 8 * f + 8], pt[:, 0:8])
                nc.scalar.copy(catT[:, 4 + cb, 8 * f : 8 * f + 8], pt[:, 8:16])
                pt2 = sp.tile([128, 8], BF, tag="psB")
                nc.tensor.transpose(pt2[:], cat_f[:, cbs], ident[0:8, 0:8])
                nc.scalar.copy(catT[:, 2 + cb, 8 * f : 8 * f + 8], pt2[:])

        # ================= fused head + LayerNorm (all 32 rows at once)
        ps_y = sp.tile([32, 256], F32, tag="roiP")
        for r in range(6):
            nc.tensor.matmul(
                ps_y[:], catT[:, r, :], fwT[:, r, :], start=(r == 0), stop=False
            )
        nc.tensor.matmul(ps_y[:], ones1[:], fb[:], start=False, stop=True)
        nc.scalar.copy(y_sb[:], ps_y[:])
        stats = tp.tile([32, 6], F32, tag="st")
        mv = tp.tile([32, 2], F32, tag="mv")
        nc.vector.bn_stats(out=stats[:], in_=y_sb[:])
        nc.vector.bn_aggr(out=mv[:], in_=stats[:])
        std = tp.tile([32, 1], F32, tag="sd")
        nc.scalar.activation(std[:], mv[:, 1:2], AF.Sqrt, bias=eps[:], scale=1.0)
        inv = tp.tile([32, 1], F32, tag="iv")
        nc.vector.reciprocal(inv[:], std[:])
        t1 = tp.tile([32, 256], F32, tag="t1")
        nc.vector.tensor_scalar(
            out=t1[:],
            in0=y_sb[:],
            scalar1=mv[:, 0:1],
            scalar2=inv[:],
            op0=SUB,
            op1=MULT,
        )
        nc.vector.tensor_tensor(out=t1[:], in0=t1[:], in1=lng[:], op=MULT)
        nc.vector.tensor_tensor(out=t1[:], in0=t1[:], in1=lnb[:], op=ADD)
        out_t = tp.tile([32, 256], F32, tag="ot")
        nc.vector.tensor_scalar_max(out_t[:], t1[:], 0.0)
        nc.sync.dma_start(out=d["out"][:], in_=out_t[:])


# ---------------------------------------------------------------- interface
_CACHE = {}


def kernel(frames, boxes, drone_mask, params):
    frames = np.asarray(frames, np.float32)
    boxes = np.asarray(boxes, np.float32)
    drone_mask = np.asarray(drone_mask, np.float32)

    consts = _pack_weights(params)
    xs = frames.reshape(N_FRAMES, 3, H, W)
    bn = boxes.reshape(N_FRAMES, M, 5)[:, :, 1:5].astype(np.float32)
    dm = drone_mask.reshape(N_FRAMES, M)
    x27, x_hw, k01, scales = _prep_frames(xs, bn, dm)

    if "nc" not in _CACHE:
        _CACHE["nc"] = _build_program()
    nc = _CACHE["nc"]

    in_maps = []
    for c in range(N_CORES):
        sl = slice(FPC * c, FPC * (c + 1))
        in_maps.append(
            {
                "x27": np.ascontiguousarray(x27[sl]),
                "x_hw": np.ascontiguousarray(x_hw[sl]),
                "k01": np.ascontiguousarray(k01[sl]),
                "scales": np.ascontiguousarray(scales[sl].T),
                **consts,
            }
        )

    res = run_bass_kernel_spmd(nc, in_maps, list(range(N_CORES)))
    outs = [res.results[c]["out"] for c in range(N_CORES)]
    full = np.concatenate(outs, axis=0)  # [8 cores * 32 rows, 256]
    return full.reshape(B, T, M, 256).astype(np.float32)
